# revision 42
# baseline (speedup 1.0000x reference)
"""Causal self-attention (sparse column mask) on 8 Trainium2 NeuronCores.

Problem: B=8, T=1024, C=512, 8 heads (hd=64).
  q/k/v = x @ W{q,k,v}.T + b;  att = softmax(mask(q k^T / 8));  y = att v
  out = y @ Wp.T + bp
Mask: causal lower-triangle, minus every column j with j % 25 == 24.

Sharding: pure data-parallel over batch — core b computes batch element b.

Per-core kernel design (all matmul operands fp16, PSUM accumulation f32):
  - Host pre-transposes x[b] -> xT [C, T]; q/k weights are packed per
    head-pair m ([P, KT*128], contiguous 1KB DMA lines) so head-pair 0's
    projections complete first and attention starts ~3 us earlier.
  - Inputs stream chunked + prioritized over three DMA queues (sync,
    gpsimd, scalar); the tensor engine issues none so matmuls start ASAP.
  - q/k projections are phase-split by T-half: half0 feeds attention ic=0
    while half1 still streams in. q bias added during PSUM evacuation
    (DVE tensor_scalar); k bias dropped (softmax shift invariance); v bias
    folded into the output bias on host (bp' = Wp @ bv + bp).
  - QK per query chunk ic (512 wide): per head-pair p, key tile J: two
    K=64 row-tiled matmuls -> S^T in PSUM; one ACT exp (scale=1/8,
    per-partition bias -30 on j%25==24 columns) -> fp16 SBUF; causal
    diagonal zeroed by one fp16 multiply with a broadcast lower-triangle
    tile (gpsimd, DVE for the last phase).
  - AV runs in [q, d] orientation (PE matmul cost = output free size, so
    M=128-query output tiles with 65-wide moving side cost 4x less than
    the [d, q] orientation + ones-matmul denominators): per (p, q-tile r,
    key tile J, head h): out av[128 q, 65] += es[:, h, qslice]^T @
    [v_h | 1]; the ones column accumulates the softmax denominator as a
    per-partition scalar for free.  Normalize = DVE reciprocal [128, 2]
    + one stride-0-broadcast tensor_tensor multiply -> yn [q, c] fp16.
  - yn is PE-transposed (128x128 blocks vs an identity) into yT [c, t]
    which feeds the unchanged output projection; one DVE evac per phase
    moves the 4 transposed blocks PSUM -> SBUF.
  - PSUM is bank-ring'd: st 2x2 banks; {pq, pv, po} ring 2x1; {pk, av
    accumulators, transpose blocks} ring 2x1 (slot lifetimes interleave
    in emission order).
  - The emission weaves qk J-units (which feed the ACT exp pipeline) with
    budget-tracked PE filler (v/out/half1 projections, AV phases) so ACT
    never starves and PE never idles; rows 4..7 finish pipelined in the
    tail (po prefire m=0..2, then per-row norm/transpose/po/evac).
"""

import numpy as np

B, T, C = 8, 1024, 512
H = 8
HD = C // H
P = 128
JD = 25  # joined dim; column j masked when j % 25 == 24
N_CORES = 8
NEG = -30.0  # added post-scale; exp(-30) flushes to 0 in fp16

_CACHE = {}


def _build():
    import concourse.bass as bass
    import concourse.mybir as mybir
    import concourse.tile as tile
    from concourse import bacc

    f16 = mybir.dt.float16
    f32 = mybir.dt.float32
    AF = mybir.ActivationFunctionType
    ALU = mybir.AluOpType

    nc = bacc.Bacc("TRN2", target_bir_lowering=False, debug=False)

    KT = C // P  # 4 c_in tiles
    MT = C // P  # 4 c_out tiles (= head pairs)
    RT = T // P  # 8 t tiles
    HT = 512  # half of T

    xT = nc.dram_tensor("xT", [C, T], f16, kind="ExternalInput").ap()
    wqm_d = [
        nc.dram_tensor(f"wqm{m}", [P, KT * P], f16, kind="ExternalInput").ap()
        for m in range(MT)
    ]
    wkm_d = [
        nc.dram_tensor(f"wkm{m}", [P, KT * P], f16, kind="ExternalInput").ap()
        for m in range(MT)
    ]
    wvT = nc.dram_tensor("wvT", [C, C], f16, kind="ExternalInput").ap()
    wpT = nc.dram_tensor("wpT", [C, C], f16, kind="ExternalInput").ap()
    # combo16: tri [:,0:128] | identity [:,128:256]; combo32: cmask [:,0:8] | bq [:,8:12]
    combo16 = nc.dram_tensor("combo16", [P, 2 * P], f16, kind="ExternalInput").ap()
    combo32 = nc.dram_tensor("combo32", [P, T // P + MT], f32, kind="ExternalInput").ap()
    bppb = nc.dram_tensor("bppb", [P, C], f16, kind="ExternalInput").ap()
    out = nc.dram_tensor("out", [T, C], f16, kind="ExternalOutput").ap()

    with tile.TileContext(nc) as tc:
        with (
            tc.tile_pool(name="const", bufs=1) as const,
            tc.tile_pool(name="persist", bufs=1) as persist,
            tc.tile_pool(name="es", bufs=24) as es_pool,
            tc.tile_pool(name="rden", bufs=8) as rden_pool,
            tc.tile_pool(name="ot", bufs=4) as ot_pool,
            tc.tile_pool(name="pbig", bufs=2, space="PSUM") as pbig,
            tc.tile_pool(name="pA", bufs=2, space="PSUM") as pA,
            tc.tile_pool(name="pB", bufs=2, space="PSUM") as pB,
        ):
            # ---- chunked, prioritized input loads over three DMA queues ----
            def load(shape, dtype, src, tag, eng):
                t = const.tile(shape, dtype, name=tag, tag=tag)
                eng.dma_start(out=t, in_=src)
                return t

            r3 = lambda a: a.rearrange("(a p) n -> p a n", p=P)  # noqa: E731
            xTr, wvr, wpr = map(r3, (xT, wvT, wpT))

            xh = [[None] * KT for _ in range(2)]
            wqm_s, wkm_s = [None] * MT, [None] * MT
            wv_c, wp_c = [None] * KT, [None] * MT
            # strict per-queue priority; sync/gpsimd carry the bulk, scalar
            # (slower queue) takes the small combos + last-pair weights
            # gpsimd's (software-DGE) queue has ~3us cold-start latency, so all
            # phase-A-critical items ride sync/scalar; gpsimd gets late bulk
            # phase-A criticals + v/x chunks ride the fast HWDGE queues (sync,
            # scalar); gpsimd (SWDGE, ~3us cold start + ~1us/issue) gets only
            # items needed after ~5us (m=2/3 weights, half-1 x, wp, bppb)
            # PE warmup: the first real matmul is DMA-gated to ~13us and the
            # tensor engine runs at a degraded pstate for ~3us after idle, so
            # spin throwaway matmuls on a memset tile during the load window
            # to enter the real stream at full clock
            dum = const.tile([P, P], f16, name="dum", tag="dum")
            nc.gpsimd.memset(dum, 0.125)

            # m0's six inputs interleave across BOTH fast queues (~0.9us per
            # 128KB chunk per queue) so phase A starts ~4us earlier; gpsimd
            # (SWDGE, slow start but then parallel) takes m2/m3 weights,
            # half-1 x, and the late wp/bppb
            xh[0][0] = load([P, 1, HT], f16, xTr[:, 0:1, 0:HT], "xh00", nc.sync)
            xh[0][1] = load([P, 1, HT], f16, xTr[:, 1:2, 0:HT], "xh01", nc.scalar)
            xh[0][2] = load([P, 1, HT], f16, xTr[:, 2:3, 0:HT], "xh02", nc.sync)
            wqm_s[0] = load([P, KT * P], f16, wqm_d[0], "wqm0", nc.scalar)
            wkm_s[0] = load([P, KT * P], f16, wkm_d[0], "wkm0", nc.sync)
            xh[0][3] = load([P, 1, HT], f16, xTr[:, 3:4, 0:HT], "xh03", nc.scalar)
            wqm_s[2] = load([P, KT * P], f16, wqm_d[2], "wqm2", nc.gpsimd)
            wqm_s[3] = load([P, KT * P], f16, wqm_d[3], "wqm3", nc.gpsimd)
            wqm_s[1] = load([P, KT * P], f16, wqm_d[1], "wqm1", nc.sync)
            cm32 = load([P, T // P + MT], f32, combo32, "c32", nc.scalar)
            wkm_s[1] = load([P, KT * P], f16, wkm_d[1], "wkm1", nc.scalar)
            wkm_s[3] = load([P, KT * P], f16, wkm_d[3], "wkm3", nc.gpsimd)
            cm16 = load([P, 2 * P], f16, combo16, "c16", nc.scalar)
            wkm_s[2] = load([P, KT * P], f16, wkm_d[2], "wkm2", nc.sync)
            wv_c[0] = load([P, 1, C], f16, wvr[:, 0:1, :], "wv0", nc.sync)
            wv_c[1] = load([P, 1, C], f16, wvr[:, 1:2, :], "wv1", nc.scalar)
            wv_c[2] = load([P, 1, C], f16, wvr[:, 2:3, :], "wv2", nc.sync)
            wv_c[3] = load([P, 1, C], f16, wvr[:, 3:4, :], "wv3", nc.scalar)
            xh[1][1] = load([P, 1, HT], f16, xTr[:, 1:2, HT:T], "xh11", nc.gpsimd)
            xh[1][0] = load([P, 1, HT], f16, xTr[:, 0:1, HT:T], "xh10", nc.sync)
            xh[1][2] = load([P, 1, HT], f16, xTr[:, 2:3, HT:T], "xh12", nc.scalar)
            xh[1][3] = load([P, 1, HT], f16, xTr[:, 3:4, HT:T], "xh13", nc.gpsimd)
            wp_c[0] = load([P, 1, C], f16, wpr[:, 0:1, :], "wp0", nc.gpsimd)
            wp_c[1] = load([P, 1, C], f16, wpr[:, 1:2, :], "wp1", nc.gpsimd)
            wp_c[2] = load([P, 1, C], f16, wpr[:, 2:3, :], "wp2", nc.gpsimd)
            wp_c[3] = load([P, 1, C], f16, wpr[:, 3:4, :], "wp3", nc.gpsimd)
            bppb_s = load([P, C], f16, bppb, "bppb", nc.sync)

            tri_ap = cm16[:, 0:P]
            ident = cm16[:, P : 2 * P]

            qT_t = [persist.tile([P, T], f16, name=f"qT{m}", tag=f"qT{m}") for m in range(MT)]
            kT_t = [persist.tile([P, T], f16, name=f"kT{m}", tag=f"kT{m}") for m in range(MT)]
            # v with a ones column per head: [v_h | 1] at cols 65h..65h+64
            v_t = [persist.tile([P, H, HD + 1], f16, name=f"v{r}", tag=f"v{r}") for r in range(RT)]
            yn_t = [persist.tile([P, MT, 2, HD], f16, name=f"yn{r}", tag=f"yn{r}") for r in range(RT)]
            yT_t = [persist.tile([P, T], f16, name=f"yT{m}", tag=f"yT{m}") for m in range(MT)]

            # ones columns of v (the denominator accumulators); DVE is idle early
            for r in range(RT):
                nc.vector.memset(v_t[r][:, :, HD : HD + 1], 1.0)

            # broadcast lower-triangle tile across both heads of an es tile
            tri_b = bass.AP(
                tensor=tri_ap.tensor,
                offset=tri_ap.offset,
                ap=[list(tri_ap.ap[0]), [0, 2], list(tri_ap.ap[1])],
            )

            # ---- emission helpers ----
            def projqk_pass(ms, h, k_on_act):
                pq = {m: pA.tile([P, HT], f32, name=f"pq{m}", tag="pA") for m in ms}
                pk = {m: pB.tile([P, HT], f32, name=f"pk{m}", tag="pB") for m in ms}
                for k in range(KT):
                    rhs = xh[h][k][:, 0, :]
                    for m in ms:
                        nc.tensor.matmul(
                            pq[m], lhsT=wqm_s[m][:, P * k : P * (k + 1)], rhs=rhs,
                            start=(k == 0), stop=(k == KT - 1),
                        )
                        nc.tensor.matmul(
                            pk[m], lhsT=wkm_s[m][:, P * k : P * (k + 1)], rhs=rhs,
                            start=(k == 0), stop=(k == KT - 1),
                        )
                for m in ms:
                    nc.vector.tensor_scalar_add(
                        qT_t[m][:, HT * h : HT * (h + 1)], pq[m],
                        cm32[:, T // P + m : T // P + m + 1],
                    )
                    if k_on_act:
                        nc.scalar.activation(
                            kT_t[m][:, HT * h : HT * (h + 1)], pk[m], AF.Copy
                        )
                    else:
                        nc.vector.tensor_copy(kT_t[m][:, HT * h : HT * (h + 1)], pk[m])

            def proj_v(r):
                h, rr = divmod(r, 4)
                ps = pA.tile([P, H, HD], f32, name="pv", tag="pA")
                for k in range(KT):
                    nc.tensor.matmul(
                        ps,
                        lhsT=xh[h][k][:, 0, P * rr : P * (rr + 1)],
                        rhs=wv_c[k][:, 0, :],
                        start=(k == 0),
                        stop=(k == KT - 1),
                    )
                nc.vector.tensor_copy(v_t[r][:, :, 0:HD], ps)

            es_t = {}

            def qk_unit(ic, p, J):
                i0 = max(512 * ic, P * J)
                w = 512 * (ic + 1) - i0
                st = pbig.tile([P, 2, 512], f32, name="st", tag="pbig")
                for h in range(2):
                    nc.tensor.matmul(
                        st[:, h, :w],
                        lhsT=kT_t[p][64 * h : 64 * (h + 1), P * J : P * (J + 1)],
                        rhs=qT_t[p][64 * h : 64 * (h + 1), i0 : i0 + w],
                        start=True,
                        stop=True,
                        tile_position=(64 * h, 0),
                    )
                es = es_pool.tile([P, 2, 512], f16, name="es", tag="es")
                es_t[(ic, p, J)] = es
                nc.scalar.activation(
                    es[:, :, :w], st[:, :, :w], AF.Exp,
                    bias=cm32[:, J : J + 1], scale=0.125,
                )
                if P * J >= 512 * ic:  # diagonal: zero the causal triangle
                    # gpsimd only once its SWDGE DMA-issue backlog has drained
                    # (~10us); everything consumed earlier goes to DVE
                    eng = nc.gpsimd if (ic == 0 and p >= 2) else nc.vector
                    eng.tensor_tensor(
                        out=es[:, :, :P], in0=es[:, :, :P], in1=tri_b, op=ALU.mult
                    )
                return w

            def av_unit(ic, p, J, lo, hi):
                # [q, d] orientation: out [128 q, 65] += es_h^T @ [v_h | 1]
                # start=True clears has_written for the WHOLE bank, so only
                # the first matmul into each bank may use it; the other
                # groups' first writes (start=False on cleared bits) then
                # overwrite-and-set per element, later ones accumulate.
                es = es_t[(ic, p, J)]
                i0 = max(512 * ic, P * J)
                r0 = 4 * ic
                for r in range(max(J, r0), r0 + 4):
                    t_ = lo if r - r0 < 2 else hi
                    ri = (r - r0) % 2
                    off = P * r - i0
                    for h in range(2):
                        nc.tensor.matmul(
                            t_[:, ri, h, 0 : HD + 1],
                            lhsT=es[:, h, off : off + P],
                            rhs=v_t[J][:, 2 * p + h, :],
                            start=(J == 0 and h == 0 and ri == 0),
                            stop=(J == r),
                            skip_group_check=True,
                        )

            def av_norm(ic, p, r, lo, hi):
                t_ = lo if r - 4 * ic < 2 else hi
                ri = (r - 4 * ic) % 2
                rd = rden_pool.tile([P, 2, 1], f32, name="rd", tag="rd")
                nc.vector.reciprocal_approx_fast(out=rd, in_=t_[:, ri, :, HD : HD + 1])
                rdb = bass.AP(
                    tensor=rd.tensor,
                    offset=rd.offset,
                    ap=[list(rd.ap[0]), list(rd.ap[1]), [0, HD]],
                )
                nc.vector.tensor_tensor(
                    out=yn_t[r][:, p],
                    in0=t_[:, ri, :, 0:HD],
                    in1=rdb,
                    op=ALU.mult,
                )

            def av_tps(ic, p):
                # transpose yn [q, c] blocks of head-pair p into yT [c, t]
                r0 = 4 * ic
                tp = pB.tile([P, 4, P], f16, name=f"tp{ic}{p}", tag="pB")
                for r in range(r0, r0 + 4):
                    nc.tensor.matmul(
                        tp[:, r - r0, :],
                        lhsT=yn_t[r][:, p],
                        rhs=ident,
                        is_transpose=True,
                    )
                nc.vector.tensor_copy(yT_t[p][:, P * r0 : P * (r0 + 4)], tp)

            def po_row(r, pool, tag):
                po = pool.tile([P, C], f32, name=f"po{r}", tag=tag)
                for m in range(MT):
                    nc.tensor.matmul(
                        po, lhsT=yT_t[m][:, P * r : P * (r + 1)], rhs=wp_c[m][:, 0, :],
                        start=(m == 0), stop=(m == MT - 1),
                    )
                finish_evac(r, po)

            def finish_evac(r, po):
                ot = ot_pool.tile([P, C], f16, name="ot", tag="ot")
                nc.vector.tensor_tensor(out=ot, in0=po, in1=bppb_s, op=ALU.add)
                eng = (nc.sync, nc.gpsimd, nc.sync, nc.gpsimd,
                       nc.sync, nc.scalar, nc.sync, nc.scalar)[r]
                eng.dma_start(out=out[P * r : P * (r + 1), :], in_=ot)

            # ---- filler stream: (est_pe_ns, closure, barrier), dependency-
            # safe order; barrier = es key that must already be emitted ----
            filler = []

            def add(est, fn, req=None):
                filler.append((est, fn, req))

            # av/po PSUM tiles must be allocated lazily (at closure run time,
            # interleaved with consumption) or the pool rings would deadlock;
            # each phase allocates inside its first closure via a mutable box.
            def add_av_phase(ic, p):
                # norm(r) is emitted AFTER unit J=r+1 (not right after its
                # stop at J=r): the DVE bank-read would otherwise serialize
                # ahead of the next unit's PE writes to the same bank
                r0 = 4 * ic
                box = {}

                for J in range(4 * (ic + 1)):
                    def unit(J=J):
                        if J == 0:
                            box["lo"] = pB.tile(
                                [P, 2, 2, HD + 1], f32, name=f"av{ic}{p}l", tag="pB"
                            )
                            box["hi"] = pB.tile(
                                [P, 2, 2, HD + 1], f32, name=f"av{ic}{p}h", tag="pB"
                            )
                        av_unit(ic, p, J, box["lo"], box["hi"])

                    nr = 4 * ic + 4 - max(J, r0)
                    add(160 * nr + 30, unit, req=(ic, p, J))
                    if J > r0:
                        add(0, lambda J=J: av_norm(ic, p, J - 1, box["lo"], box["hi"]))
                add(0, lambda: av_norm(ic, p, r0 + 3, box["lo"], box["hi"]))
                add(450, lambda: av_tps(ic, p))

            def inline_av_phase(ic, p):
                r0 = 4 * ic
                lo = pB.tile([P, 2, 2, HD + 1], f32, name=f"av{ic}{p}l", tag="pB")
                hi = pB.tile([P, 2, 2, HD + 1], f32, name=f"av{ic}{p}h", tag="pB")
                for J in range(4 * (ic + 1)):
                    av_unit(ic, p, J, lo, hi)
                    if J > r0:
                        av_norm(ic, p, J - 1, lo, hi)
                av_norm(ic, p, r0 + 3, lo, hi)
                av_tps(ic, p)

            def add_B(m):
                # half1 q/k projection for head-pair m, split into two
                # 4-matmul closures (k=0,1 then k=2,3) for finer weave grain
                box = {}

                def part(ks):
                    if ks[0] == 0:
                        box["pq"] = pA.tile([P, HT], f32, name=f"pqB{m}", tag="pA")
                        box["pk"] = pB.tile([P, HT], f32, name=f"pkB{m}", tag="pB")
                    for k in ks:
                        rhs = xh[1][k][:, 0, :]
                        nc.tensor.matmul(
                            box["pq"], lhsT=wqm_s[m][:, P * k : P * (k + 1)], rhs=rhs,
                            start=(k == 0), stop=(k == KT - 1),
                        )
                        nc.tensor.matmul(
                            box["pk"], lhsT=wkm_s[m][:, P * k : P * (k + 1)], rhs=rhs,
                            start=(k == 0), stop=(k == KT - 1),
                        )
                    if ks[-1] == KT - 1:
                        nc.vector.tensor_scalar_add(
                            qT_t[m][:, HT:T], box["pq"],
                            cm32[:, T // P + m : T // P + m + 1],
                        )
                        nc.vector.tensor_copy(kT_t[m][:, HT:T], box["pk"])

                add(1050, lambda: part((0, 1)))
                add(1050, lambda: part((2, 3)))

            # v0-3 and av(0,0) only need half-0 inputs and early es tiles and
            # so fill the hole while the slower half-1 x chunks stream in
            for r in (0, 1, 2, 3):
                add(1050, lambda r=r: proj_v(r))
            add_av_phase(0, 0)
            add_B(0)
            add_B(1)
            add_av_phase(0, 1)
            for r in (4, 5, 6, 7):
                add(1050, lambda r=r: proj_v(r))
            add_B(2)
            add_av_phase(0, 2)
            add_B(3)
            add_av_phase(0, 3)
            add_av_phase(1, 0)
            add_av_phase(1, 1)
            add_av_phase(1, 2)

            # ---- emission ----
            dum_ps = pA.tile([P, C], f32, name="dumps", tag="pA")
            for _ in range(56):
                nc.tensor.matmul(dum_ps[:, 0:P], lhsT=dum, rhs=dum, start=True, stop=True)

            # phase A, m-minor, with qk(0,0)/(0,1) J-pairs interleaved so the
            # first exp fires as soon as head-pair 0's projections land
            projqk_pass((0,), 0, True)
            qk_unit(0, 0, 0)
            qk_unit(0, 0, 1)
            projqk_pass((1,), 0, False)
            qk_unit(0, 0, 2)
            qk_unit(0, 0, 3)
            projqk_pass((2,), 0, False)
            qk_unit(0, 1, 0)
            qk_unit(0, 1, 1)
            projqk_pass((3,), 0, False)
            qk_unit(0, 1, 2)
            qk_unit(0, 1, 3)

            # act starts at the estimated END of the 8 phase-A-interleaved
            # exps (first exp ~4.2us + 8 units), so budgets model real ACT
            state = {"pe": 0.0, "act": 11800.0, "fi": 0}

            def fill(upto):
                while state["fi"] < len(filler) and state["pe"] < upto:
                    est, fn, req = filler[state["fi"]]
                    if req is not None and req not in es_t:
                        break
                    fn()
                    state["pe"] += est
                    state["fi"] += 1

            # emit qk units in J-pairs (matches the 2-deep st ring); filler is
            # emitted BEFORE each quantum so the PE stream arrives at qk n
            # roughly when exp n-2 frees its st slot
            for ic, p in ((0, 2), (0, 3), (1, 0), (1, 1), (1, 2), (1, 3)):
                quanta = ((0, 1), (2, 3)) if ic == 0 else ((0, 1), (2, 3), (4, 5), (6, 7))
                for q in quanta:
                    qa = sum(
                        2.2 * (512 * (ic + 1) - max(512 * ic, P * J)) + 250
                        for J in q
                    )
                    fill(state["act"] - 400)
                    for J in q:
                        w = qk_unit(ic, p, J)
                        state["pe"] += 0.84 * w + 60
                    state["act"] = max(state["act"], state["pe"]) + qa
            fill(float("inf"))

            # ---- tail ----
            # sized to match ACT's drain of the final exp backlog (~8us):
            # po rows 0..3, po 4/5 prefire, then av(1,3).  The po67 prefire
            # (whose st-pool slot is freed by the next-to-last exp) is
            # deferred past the early av(1,3) units so its stall doesn't
            # block them in the PE queue.
            for r in (0, 1, 2, 3):
                po_row(r, pA, "pA")
            po4 = pA.tile([P, C], f32, name="po4", tag="pA")
            po5 = pA.tile([P, C], f32, name="po5", tag="pA")
            po_tail = {4: po4, 5: po5}

            def prefire(r):
                for m in range(3):
                    nc.tensor.matmul(
                        po_tail[r], lhsT=yT_t[m][:, P * r : P * (r + 1)],
                        rhs=wp_c[m][:, 0, :], start=(m == 0), stop=False,
                    )
                # fold the output bias in as a K=1 rank-1 matmul (ones ⊗ bpp)
                # so the tail evac is a plain copy the idle ACT engine can do
                nc.tensor.matmul(
                    po_tail[r], lhsT=tri_ap[0:1, :], rhs=bppb_s[0:1, :],
                    start=False, stop=False,
                )

            prefire(4)
            prefire(5)

            # av phase (1, 3): per-row norm -> transpose -> po m=3 -> out
            # rows 4-6 transpose via the DMA xbar (SBUF->SBUF, no PSUM needed);
            # row 7 via PE into the st slot freed by the final exp
            lo13 = pB.tile([P, 2, 2, HD + 1], f32, name="av13l", tag="pB")
            hi13 = pB.tile([P, 2, 2, HD + 1], f32, name="av13h", tag="pB")

            tp67 = {}

            def tail_row(r):
                av_norm(1, 3, r, lo13, hi13)
                if r < 6:
                    # sync queue: a DMA issue on the scalar queue would block
                    # the ACT sequencer ~667ns mid-exp-stream; gpsimd is
                    # SWDGE (no xbar access)
                    nc.sync.dma_start_transpose(
                        out=yT_t[3][:, P * r : P * (r + 1)],
                        in_=yn_t[r][:, 3],
                    )
                else:
                    # rows 6/7: PE transpose into the st slot freed by the
                    # final exp (faster than the ~1.2us xbar DMA round trip)
                    if "t" not in tp67:
                        tp67["t"] = pbig.tile([P, 2, P], f16, name="tp67", tag="pbig")
                    tp = tp67["t"][:, r - 6, :]
                    nc.tensor.matmul(
                        tp, lhsT=yn_t[r][:, 3], rhs=ident, is_transpose=True
                    )
                    nc.vector.tensor_copy(yT_t[3][:, P * r : P * (r + 1)], tp)
                nc.tensor.matmul(
                    po_tail[r], lhsT=yT_t[3][:, P * r : P * (r + 1)],
                    rhs=wp_c[3][:, 0, :], start=False, stop=True,
                )
                ot = ot_pool.tile([P, C], f16, name="ot", tag="ot")
                nc.scalar.activation(ot, po_tail[r], AF.Copy)
                eng = (nc.sync, nc.gpsimd, nc.sync, nc.scalar)[r - 4]
                eng.dma_start(out=out[P * r : P * (r + 1), :], in_=ot)

            for J in range(6):
                av_unit(1, 3, J, lo13, hi13)
            tail_row(4)
            po67 = pbig.tile([P, 2, 512], f32, name="po67", tag="pbig")
            po_tail[6] = po67[:, 0, :]
            po_tail[7] = po67[:, 1, :]
            prefire(6)
            prefire(7)
            av_unit(1, 3, 6, lo13, hi13)
            tail_row(5)
            av_unit(1, 3, 7, lo13, hi13)
            tail_row(6)
            tail_row(7)

    nc.compile()
    return nc


def _prep_inputs(x, Wq, bq, Wk, bk, Wv, bv, Wp, bp):
    """Host-side prep: transposes, bias folding, mask tables. Returns in_maps."""
    f16 = np.float16
    wqT = np.ascontiguousarray(Wq.T).astype(f16)
    wkT = np.ascontiguousarray(Wk.T).astype(f16)
    wvT = np.ascontiguousarray(Wv.T).astype(f16)
    wpT = np.ascontiguousarray(Wp.T).astype(f16)

    def mpack(wT, m):  # [P, KT*P]: (p, k*128+j) -> wT[128k+p, 128m+j]
        return np.ascontiguousarray(
            wT.reshape(C // P, P, C)[:, :, P * m : P * (m + 1)].transpose(1, 0, 2)
        ).reshape(P, C)

    bq_pp = np.ascontiguousarray(bq.astype(np.float32).reshape(C // P, P).T)
    # v bias folds into output bias: out = (y' + bv) @ Wp.T + bp
    bpp = (
        Wp.astype(np.float64) @ bv.astype(np.float64) + bp.astype(np.float64)
    ).astype(np.float32)
    bppb = np.broadcast_to(bpp[None, :], (P, C)).astype(f16).copy()
    ident = np.eye(P, dtype=f16)
    tri = (np.arange(P)[:, None] <= np.arange(P)[None, :]).astype(f16)  # keep j<=i
    j_idx = np.arange(P)[:, None] + P * np.arange(T // P)[None, :]
    cmask = np.where(j_idx % JD == JD - 1, np.float32(NEG), np.float32(0.0)).astype(
        np.float32
    )

    shared = {
        "wvT": wvT,
        "wpT": wpT,
        "combo16": np.ascontiguousarray(np.concatenate([tri, ident], axis=1)),
        "combo32": np.ascontiguousarray(
            np.concatenate([cmask, bq_pp], axis=1).astype(np.float32)
        ),
        "bppb": bppb,
    }
    for m in range(C // P):
        shared[f"wqm{m}"] = mpack(wqT, m)
        shared[f"wkm{m}"] = mpack(wkT, m)
    in_maps = []
    for b in range(N_CORES):
        mm = dict(shared)
        mm["xT"] = np.ascontiguousarray(x[b].T).astype(f16)
        in_maps.append(mm)
    return in_maps


def kernel(x, Wq, bq, Wk, bk, Wv, bv, Wp, bp):
    from concourse import bass_utils

    x = np.asarray(x, dtype=np.float32)
    if "nc" not in _CACHE:
        _CACHE["nc"] = _build()
    nc = _CACHE["nc"]
    in_maps = _prep_inputs(
        x,
        np.asarray(Wq, np.float32),
        np.asarray(bq, np.float32),
        np.asarray(Wk, np.float32),
        np.asarray(bk, np.float32),
        np.asarray(Wv, np.float32),
        np.asarray(bv, np.float32),
        np.asarray(Wp, np.float32),
        np.asarray(bp, np.float32),
    )
    res = bass_utils.run_bass_kernel_spmd(nc, in_maps, core_ids=list(range(N_CORES)))
    return np.stack(
        [res.results[b]["out"].astype(np.float32) for b in range(N_CORES)], axis=0
    )


# revision 44
# speedup vs baseline: 1.0693x; 1.0693x over previous
"""Causal self-attention (sparse column mask) on 8 Trainium2 NeuronCores.

Problem: B=8, T=1024, C=512, 8 heads (hd=64).
  q/k/v = x @ W{q,k,v}.T + b;  att = softmax(mask(q k^T / 8));  y = att v
  out = y @ Wp.T + bp
Mask: causal lower-triangle, minus every column j with j % 25 == 24.

Sharding: pure data-parallel over batch — core b computes batch element b.

Per-core kernel design (all matmul operands fp16, PSUM accumulation f32):
  - Host pre-transposes x[b] -> xT [C, T]; q/k weights are packed per
    head-pair m ([P, KT*128], contiguous 1KB DMA lines) so head-pair 0's
    projections complete first and attention starts ~3 us earlier.
  - Inputs stream chunked + prioritized over three DMA queues (sync,
    gpsimd, scalar); the tensor engine issues none so matmuls start ASAP.
  - q/k projections are phase-split by T-half: half0 feeds attention ic=0
    while half1 still streams in. q bias added during PSUM evacuation
    (DVE tensor_scalar); k bias dropped (softmax shift invariance); v bias
    folded into the output bias on host (bp' = Wp @ bv + bp).
  - QK per query chunk ic (512 wide): per head-pair p, key tile J: two
    K=64 row-tiled matmuls -> S^T in PSUM; one ACT exp (scale=1/8,
    per-partition bias -30 on j%25==24 columns) -> fp16 SBUF; causal
    diagonal zeroed by one fp16 multiply with a broadcast lower-triangle
    tile (gpsimd, DVE for the last phase).
  - AV runs in [q, d] orientation (PE matmul cost = output free size, so
    M=128-query output tiles with 65-wide moving side cost 4x less than
    the [d, q] orientation + ones-matmul denominators): per (p, q-tile r,
    key tile J, head h): out av[128 q, 65] += es[:, h, qslice]^T @
    [v_h | 1]; the ones column accumulates the softmax denominator as a
    per-partition scalar for free.  Normalize = DVE reciprocal [128, 2]
    + one stride-0-broadcast tensor_tensor multiply -> yn [q, c] fp16.
  - yn is PE-transposed (128x128 blocks vs an identity) into yT [c, t]
    which feeds the unchanged output projection; one DVE evac per phase
    moves the 4 transposed blocks PSUM -> SBUF.
  - PSUM is bank-ring'd: st 2x2 banks; {pq, pv, po} ring 2x1; {pk, av
    accumulators, transpose blocks} ring 2x1 (slot lifetimes interleave
    in emission order).
  - The emission weaves qk J-units (which feed the ACT exp pipeline) with
    budget-tracked PE filler (v/out/half1 projections, AV phases) so ACT
    never starves and PE never idles; rows 4..7 finish pipelined in the
    tail (po prefire m=0..2, then per-row norm/transpose/po/evac).
"""

import numpy as np

B, T, C = 8, 1024, 512
H = 8
HD = C // H
P = 128
JD = 25  # joined dim; column j masked when j % 25 == 24
N_CORES = 8
NEG = -30.0  # added post-scale; exp(-30) flushes to 0 in fp16

_CACHE = {}


def _build():
    import concourse.bass as bass
    import concourse.mybir as mybir
    import concourse.tile as tile
    from concourse import bacc

    f16 = mybir.dt.float16
    f32 = mybir.dt.float32
    AF = mybir.ActivationFunctionType
    ALU = mybir.AluOpType

    nc = bacc.Bacc("TRN2", target_bir_lowering=False, debug=False)

    KT = C // P  # 4 c_in tiles
    MT = C // P  # 4 c_out tiles (= head pairs)
    RT = T // P  # 8 t tiles
    HT = 512  # half of T

    xT = nc.dram_tensor("xT", [C, T], f16, kind="ExternalInput").ap()
    wqm_d = [
        nc.dram_tensor(f"wqm{m}", [P, KT * P], f16, kind="ExternalInput").ap()
        for m in range(MT)
    ]
    wkm_d = [
        nc.dram_tensor(f"wkm{m}", [P, KT * P], f16, kind="ExternalInput").ap()
        for m in range(MT)
    ]
    wvT = nc.dram_tensor("wvT", [C, C], f16, kind="ExternalInput").ap()
    wpT = nc.dram_tensor("wpT", [C, C], f16, kind="ExternalInput").ap()
    # combo16: tri [:,0:128] | identity [:,128:256]; combo32: cmask [:,0:8] | bq [:,8:12]
    combo16 = nc.dram_tensor("combo16", [P, 2 * P], f16, kind="ExternalInput").ap()
    combo32 = nc.dram_tensor("combo32", [P, T // P + MT], f32, kind="ExternalInput").ap()
    bppb = nc.dram_tensor("bppb", [P, C], f16, kind="ExternalInput").ap()
    out = nc.dram_tensor("out", [T, C], f16, kind="ExternalOutput").ap()

    with tile.TileContext(nc) as tc:
        with (
            tc.tile_pool(name="const", bufs=1) as const,
            tc.tile_pool(name="persist", bufs=1) as persist,
            tc.tile_pool(name="es", bufs=24) as es_pool,
            tc.tile_pool(name="rden", bufs=8) as rden_pool,
            tc.tile_pool(name="ot", bufs=4) as ot_pool,
            tc.tile_pool(name="pbig", bufs=2, space="PSUM") as pbig,
            tc.tile_pool(name="pA", bufs=2, space="PSUM") as pA,
            tc.tile_pool(name="pB", bufs=2, space="PSUM") as pB,
        ):
            # ---- chunked, prioritized input loads over three DMA queues ----
            def load(shape, dtype, src, tag, eng):
                t = const.tile(shape, dtype, name=tag, tag=tag)
                eng.dma_start(out=t, in_=src)
                return t

            r3 = lambda a: a.rearrange("(a p) n -> p a n", p=P)  # noqa: E731
            xTr, wvr, wpr = map(r3, (xT, wvT, wpT))

            xh = [[None] * KT for _ in range(2)]
            wqm_s, wkm_s = [None] * MT, [None] * MT
            wv_c, wp_c = [None] * KT, [None] * MT
            # strict per-queue priority; sync/gpsimd carry the bulk, scalar
            # (slower queue) takes the small combos + last-pair weights
            # gpsimd's (software-DGE) queue has ~3us cold-start latency, so all
            # phase-A-critical items ride sync/scalar; gpsimd gets late bulk
            # phase-A criticals + v/x chunks ride the fast HWDGE queues (sync,
            # scalar); gpsimd (SWDGE, ~3us cold start + ~1us/issue) gets only
            # items needed after ~5us (m=2/3 weights, half-1 x, wp, bppb)
            # PE warmup: the first real matmul is DMA-gated to ~13us and the
            # tensor engine runs at a degraded pstate for ~3us after idle, so
            # spin throwaway matmuls on a memset tile during the load window
            # to enter the real stream at full clock
            dum = const.tile([P, P], f16, name="dum", tag="dum")
            nc.gpsimd.memset(dum, 0.125)

            # m0's six inputs interleave across BOTH fast queues (~0.9us per
            # 128KB chunk per queue) so phase A starts ~4us earlier; gpsimd
            # (SWDGE, slow start but then parallel) takes m2/m3 weights,
            # half-1 x, and the late wp/bppb
            xh[0][0] = load([P, 1, HT], f16, xTr[:, 0:1, 0:HT], "xh00", nc.sync)
            xh[0][1] = load([P, 1, HT], f16, xTr[:, 1:2, 0:HT], "xh01", nc.scalar)
            xh[0][2] = load([P, 1, HT], f16, xTr[:, 2:3, 0:HT], "xh02", nc.sync)
            wqm_s[0] = load([P, KT * P], f16, wqm_d[0], "wqm0", nc.scalar)
            wkm_s[0] = load([P, KT * P], f16, wkm_d[0], "wkm0", nc.sync)
            xh[0][3] = load([P, 1, HT], f16, xTr[:, 3:4, 0:HT], "xh03", nc.scalar)
            wqm_s[2] = load([P, KT * P], f16, wqm_d[2], "wqm2", nc.gpsimd)
            wqm_s[3] = load([P, KT * P], f16, wqm_d[3], "wqm3", nc.gpsimd)
            wqm_s[1] = load([P, KT * P], f16, wqm_d[1], "wqm1", nc.sync)
            wkm_s[1] = load([P, KT * P], f16, wkm_d[1], "wkm1", nc.scalar)
            cm32 = load([P, T // P + MT], f32, combo32, "c32", nc.scalar)
            wkm_s[3] = load([P, KT * P], f16, wkm_d[3], "wkm3", nc.gpsimd)
            cm16 = load([P, 2 * P], f16, combo16, "c16", nc.scalar)
            wkm_s[2] = load([P, KT * P], f16, wkm_d[2], "wkm2", nc.sync)
            wv_c[0] = load([P, 1, C], f16, wvr[:, 0:1, :], "wv0", nc.sync)
            wv_c[1] = load([P, 1, C], f16, wvr[:, 1:2, :], "wv1", nc.scalar)
            wv_c[2] = load([P, 1, C], f16, wvr[:, 2:3, :], "wv2", nc.sync)
            wv_c[3] = load([P, 1, C], f16, wvr[:, 3:4, :], "wv3", nc.scalar)
            xh[1][1] = load([P, 1, HT], f16, xTr[:, 1:2, HT:T], "xh11", nc.gpsimd)
            xh[1][0] = load([P, 1, HT], f16, xTr[:, 0:1, HT:T], "xh10", nc.sync)
            xh[1][2] = load([P, 1, HT], f16, xTr[:, 2:3, HT:T], "xh12", nc.scalar)
            xh[1][3] = load([P, 1, HT], f16, xTr[:, 3:4, HT:T], "xh13", nc.gpsimd)
            wp_c[0] = load([P, 1, C], f16, wpr[:, 0:1, :], "wp0", nc.gpsimd)
            wp_c[1] = load([P, 1, C], f16, wpr[:, 1:2, :], "wp1", nc.gpsimd)
            wp_c[2] = load([P, 1, C], f16, wpr[:, 2:3, :], "wp2", nc.gpsimd)
            wp_c[3] = load([P, 1, C], f16, wpr[:, 3:4, :], "wp3", nc.gpsimd)
            bppb_s = load([P, C], f16, bppb, "bppb", nc.sync)

            tri_ap = cm16[:, 0:P]
            ident = cm16[:, P : 2 * P]

            qT_t = [persist.tile([P, T], f16, name=f"qT{m}", tag=f"qT{m}") for m in range(MT)]
            kT_t = [persist.tile([P, T], f16, name=f"kT{m}", tag=f"kT{m}") for m in range(MT)]
            # v with a ones column per head: [v_h | 1] at cols 65h..65h+64
            v_t = [persist.tile([P, H, HD + 1], f16, name=f"v{r}", tag=f"v{r}") for r in range(RT)]
            yn_t = [persist.tile([P, MT, 2, HD], f16, name=f"yn{r}", tag=f"yn{r}") for r in range(RT)]
            yT_t = [persist.tile([P, T], f16, name=f"yT{m}", tag=f"yT{m}") for m in range(MT)]

            # ones columns of v (the denominator accumulators); DVE is idle early
            for r in range(RT):
                nc.vector.memset(v_t[r][:, :, HD : HD + 1], 1.0)

            # broadcast lower-triangle tile across both heads of an es tile
            tri_b = bass.AP(
                tensor=tri_ap.tensor,
                offset=tri_ap.offset,
                ap=[list(tri_ap.ap[0]), [0, 2], list(tri_ap.ap[1])],
            )

            # ---- emission helpers ----
            def projqk_pass(ms, h, k_on_act):
                pq = {m: pA.tile([P, HT], f32, name=f"pq{m}", tag="pA") for m in ms}
                pk = {m: pB.tile([P, HT], f32, name=f"pk{m}", tag="pB") for m in ms}
                for k in range(KT):
                    rhs = xh[h][k][:, 0, :]
                    for m in ms:
                        nc.tensor.matmul(
                            pq[m], lhsT=wqm_s[m][:, P * k : P * (k + 1)], rhs=rhs,
                            start=(k == 0), stop=(k == KT - 1),
                        )
                        nc.tensor.matmul(
                            pk[m], lhsT=wkm_s[m][:, P * k : P * (k + 1)], rhs=rhs,
                            start=(k == 0), stop=(k == KT - 1),
                        )
                for m in ms:
                    nc.vector.tensor_scalar_add(
                        qT_t[m][:, HT * h : HT * (h + 1)], pq[m],
                        cm32[:, T // P + m : T // P + m + 1],
                    )
                    if k_on_act:
                        nc.scalar.activation(
                            kT_t[m][:, HT * h : HT * (h + 1)], pk[m], AF.Copy
                        )
                    else:
                        nc.vector.tensor_copy(kT_t[m][:, HT * h : HT * (h + 1)], pk[m])

            def proj_v(r):
                h, rr = divmod(r, 4)
                ps = pA.tile([P, H, HD], f32, name="pv", tag="pA")
                for k in range(KT):
                    nc.tensor.matmul(
                        ps,
                        lhsT=xh[h][k][:, 0, P * rr : P * (rr + 1)],
                        rhs=wv_c[k][:, 0, :],
                        start=(k == 0),
                        stop=(k == KT - 1),
                    )
                nc.vector.tensor_copy(v_t[r][:, :, 0:HD], ps)

            es_t = {}

            def qk_unit(ic, p, J):
                i0 = max(512 * ic, P * J)
                w = 512 * (ic + 1) - i0
                st = pbig.tile([P, 2, 512], f32, name="st", tag="pbig")
                for h in range(2):
                    nc.tensor.matmul(
                        st[:, h, :w],
                        lhsT=kT_t[p][64 * h : 64 * (h + 1), P * J : P * (J + 1)],
                        rhs=qT_t[p][64 * h : 64 * (h + 1), i0 : i0 + w],
                        start=True,
                        stop=True,
                        tile_position=(64 * h, 0),
                    )
                es = es_pool.tile([P, 2, 512], f16, name="es", tag="es")
                es_t[(ic, p, J)] = es
                nc.scalar.activation(
                    es[:, :, :w], st[:, :, :w], AF.Exp,
                    bias=cm32[:, J : J + 1], scale=0.125,
                )
                if P * J >= 512 * ic:  # diagonal: zero the causal triangle
                    # gpsimd only once its SWDGE DMA-issue backlog has drained
                    # (~10us); everything consumed earlier goes to DVE
                    eng = nc.gpsimd if (ic == 0 and p >= 2) else nc.vector
                    eng.tensor_tensor(
                        out=es[:, :, :P], in0=es[:, :, :P], in1=tri_b, op=ALU.mult
                    )
                return w

            def av_unit(ic, p, J, lo, hi):
                # [q, d] orientation: out [128 q, 65] += es_h^T @ [v_h | 1]
                # start=True clears has_written for the WHOLE bank, so only
                # the first matmul into each bank may use it; the other
                # groups' first writes (start=False on cleared bits) then
                # overwrite-and-set per element, later ones accumulate.
                es = es_t[(ic, p, J)]
                i0 = max(512 * ic, P * J)
                r0 = 4 * ic
                for r in range(max(J, r0), r0 + 4):
                    t_ = lo if r - r0 < 2 else hi
                    ri = (r - r0) % 2
                    off = P * r - i0
                    for h in range(2):
                        nc.tensor.matmul(
                            t_[:, ri, h, 0 : HD + 1],
                            lhsT=es[:, h, off : off + P],
                            rhs=v_t[J][:, 2 * p + h, :],
                            start=(J == 0 and h == 0 and ri == 0),
                            stop=(J == r),
                            skip_group_check=True,
                        )

            def av_norm(ic, p, r, lo, hi):
                t_ = lo if r - 4 * ic < 2 else hi
                ri = (r - 4 * ic) % 2
                rd = rden_pool.tile([P, 2, 1], f32, name="rd", tag="rd")
                nc.vector.reciprocal_approx_fast(out=rd, in_=t_[:, ri, :, HD : HD + 1])
                rdb = bass.AP(
                    tensor=rd.tensor,
                    offset=rd.offset,
                    ap=[list(rd.ap[0]), list(rd.ap[1]), [0, HD]],
                )
                nc.vector.tensor_tensor(
                    out=yn_t[r][:, p],
                    in0=t_[:, ri, :, 0:HD],
                    in1=rdb,
                    op=ALU.mult,
                )

            def av_tps(ic, p):
                # transpose yn [q, c] blocks of head-pair p into yT [c, t]
                r0 = 4 * ic
                tp = pB.tile([P, 4, P], f16, name=f"tp{ic}{p}", tag="pB")
                for r in range(r0, r0 + 4):
                    nc.tensor.matmul(
                        tp[:, r - r0, :],
                        lhsT=yn_t[r][:, p],
                        rhs=ident,
                        is_transpose=True,
                    )
                nc.vector.tensor_copy(yT_t[p][:, P * r0 : P * (r0 + 4)], tp)

            def po_row(r, pool, tag):
                po = pool.tile([P, C], f32, name=f"po{r}", tag=tag)
                for m in range(MT):
                    nc.tensor.matmul(
                        po, lhsT=yT_t[m][:, P * r : P * (r + 1)], rhs=wp_c[m][:, 0, :],
                        start=(m == 0), stop=(m == MT - 1),
                    )
                finish_evac(r, po)

            def finish_evac(r, po):
                ot = ot_pool.tile([P, C], f16, name="ot", tag="ot")
                nc.vector.tensor_tensor(out=ot, in0=po, in1=bppb_s, op=ALU.add)
                eng = (nc.sync, nc.gpsimd, nc.sync, nc.gpsimd,
                       nc.sync, nc.scalar, nc.sync, nc.scalar)[r]
                eng.dma_start(out=out[P * r : P * (r + 1), :], in_=ot)

            # ---- filler stream: (est_pe_ns, closure, barrier), dependency-
            # safe order; barrier = es key that must already be emitted ----
            filler = []

            def add(est, fn, req=None):
                filler.append((est, fn, req))

            # av/po PSUM tiles must be allocated lazily (at closure run time,
            # interleaved with consumption) or the pool rings would deadlock;
            # each phase allocates inside its first closure via a mutable box.
            def add_av_phase(ic, p):
                # norm(r) is emitted AFTER unit J=r+1 (not right after its
                # stop at J=r): the DVE bank-read would otherwise serialize
                # ahead of the next unit's PE writes to the same bank
                r0 = 4 * ic
                box = {}

                for J in range(4 * (ic + 1)):
                    def unit(J=J):
                        if J == 0:
                            box["lo"] = pB.tile(
                                [P, 2, 2, HD + 1], f32, name=f"av{ic}{p}l", tag="pB"
                            )
                            box["hi"] = pB.tile(
                                [P, 2, 2, HD + 1], f32, name=f"av{ic}{p}h", tag="pB"
                            )
                        av_unit(ic, p, J, box["lo"], box["hi"])

                    nr = 4 * ic + 4 - max(J, r0)
                    add(160 * nr + 30, unit, req=(ic, p, J))
                    if J > r0:
                        add(0, lambda J=J: av_norm(ic, p, J - 1, box["lo"], box["hi"]))
                add(0, lambda: av_norm(ic, p, r0 + 3, box["lo"], box["hi"]))
                add(450, lambda: av_tps(ic, p))

            def inline_av_phase(ic, p):
                r0 = 4 * ic
                lo = pB.tile([P, 2, 2, HD + 1], f32, name=f"av{ic}{p}l", tag="pB")
                hi = pB.tile([P, 2, 2, HD + 1], f32, name=f"av{ic}{p}h", tag="pB")
                for J in range(4 * (ic + 1)):
                    av_unit(ic, p, J, lo, hi)
                    if J > r0:
                        av_norm(ic, p, J - 1, lo, hi)
                av_norm(ic, p, r0 + 3, lo, hi)
                av_tps(ic, p)

            def add_B(m):
                # half1 q/k projection for head-pair m, split into two
                # 4-matmul closures (k=0,1 then k=2,3) for finer weave grain
                box = {}

                def part(ks):
                    if ks[0] == 0:
                        box["pq"] = pA.tile([P, HT], f32, name=f"pqB{m}", tag="pA")
                        box["pk"] = pB.tile([P, HT], f32, name=f"pkB{m}", tag="pB")
                    for k in ks:
                        rhs = xh[1][k][:, 0, :]
                        nc.tensor.matmul(
                            box["pq"], lhsT=wqm_s[m][:, P * k : P * (k + 1)], rhs=rhs,
                            start=(k == 0), stop=(k == KT - 1),
                        )
                        nc.tensor.matmul(
                            box["pk"], lhsT=wkm_s[m][:, P * k : P * (k + 1)], rhs=rhs,
                            start=(k == 0), stop=(k == KT - 1),
                        )
                    if ks[-1] == KT - 1:
                        nc.vector.tensor_scalar_add(
                            qT_t[m][:, HT:T], box["pq"],
                            cm32[:, T // P + m : T // P + m + 1],
                        )
                        nc.vector.tensor_copy(kT_t[m][:, HT:T], box["pk"])

                add(1050, lambda: part((0, 1)))
                add(1050, lambda: part((2, 3)))

            # v0-3 and av(0,0) only need half-0 inputs and early es tiles and
            # so fill the hole while the slower half-1 x chunks stream in
            for r in (0, 1, 2, 3):
                add(1050, lambda r=r: proj_v(r))
            add_av_phase(0, 0)
            add_B(0)
            add_B(1)
            add_av_phase(0, 1)
            for r in (4, 5, 6, 7):
                add(1050, lambda r=r: proj_v(r))
            add_B(2)
            add_av_phase(0, 2)
            add_B(3)
            add_av_phase(0, 3)
            add_av_phase(1, 0)
            add_av_phase(1, 1)
            add_av_phase(1, 2)

            # ---- emission ----
            dum_ps = pA.tile([P, C], f32, name="dumps", tag="pA")
            for _ in range(48):
                nc.tensor.matmul(dum_ps[:, 0:P], lhsT=dum, rhs=dum, start=True, stop=True)

            # phase A, m-minor, with qk(0,0)/(0,1) J-pairs interleaved so the
            # first exp fires as soon as head-pair 0's projections land
            projqk_pass((0,), 0, True)
            qk_unit(0, 0, 0)
            qk_unit(0, 0, 1)
            projqk_pass((1,), 0, False)
            qk_unit(0, 0, 2)
            qk_unit(0, 0, 3)
            projqk_pass((2,), 0, False)
            qk_unit(0, 1, 0)
            qk_unit(0, 1, 1)
            projqk_pass((3,), 0, False)
            qk_unit(0, 1, 2)
            qk_unit(0, 1, 3)

            # act starts at the estimated END of the 8 phase-A-interleaved
            # exps (first exp ~4.2us + 8 units), so budgets model real ACT
            state = {"pe": 0.0, "act": 11800.0, "fi": 0}

            def fill(upto):
                while state["fi"] < len(filler) and state["pe"] < upto:
                    est, fn, req = filler[state["fi"]]
                    if req is not None and req not in es_t:
                        break
                    fn()
                    state["pe"] += est
                    state["fi"] += 1

            # emit qk units in J-pairs (matches the 2-deep st ring); filler is
            # emitted BEFORE each quantum so the PE stream arrives at qk n
            # roughly when exp n-2 frees its st slot
            for ic, p in ((0, 2), (0, 3), (1, 0), (1, 1), (1, 2), (1, 3)):
                quanta = ((0, 1), (2, 3)) if ic == 0 else ((0, 1), (2, 3), (4, 5), (6, 7))
                for q in quanta:
                    qa = sum(
                        2.2 * (512 * (ic + 1) - max(512 * ic, P * J)) + 250
                        for J in q
                    )
                    fill(state["act"] - 400)
                    for J in q:
                        w = qk_unit(ic, p, J)
                        state["pe"] += 0.84 * w + 60
                    state["act"] = max(state["act"], state["pe"]) + qa
            fill(float("inf"))

            # ---- tail ----
            # sized to match ACT's drain of the final exp backlog (~8us):
            # po rows 0..3, po 4/5 prefire, then av(1,3).  The po67 prefire
            # (whose st-pool slot is freed by the next-to-last exp) is
            # deferred past the early av(1,3) units so its stall doesn't
            # block them in the PE queue.
            for r in (0, 1, 2, 3):
                po_row(r, pA, "pA")
            po4 = pA.tile([P, C], f32, name="po4", tag="pA")
            po5 = pA.tile([P, C], f32, name="po5", tag="pA")
            po_tail = {4: po4, 5: po5}

            def prefire(r):
                for m in range(3):
                    nc.tensor.matmul(
                        po_tail[r], lhsT=yT_t[m][:, P * r : P * (r + 1)],
                        rhs=wp_c[m][:, 0, :], start=(m == 0), stop=False,
                    )
                # fold the output bias in as a K=1 rank-1 matmul (ones ⊗ bpp)
                # so the tail evac is a plain copy the idle ACT engine can do
                nc.tensor.matmul(
                    po_tail[r], lhsT=tri_ap[0:1, :], rhs=bppb_s[0:1, :],
                    start=False, stop=False,
                )

            prefire(4)
            prefire(5)

            # av phase (1, 3): per-row norm -> transpose -> po m=3 -> out
            # rows 4-6 transpose via the DMA xbar (SBUF->SBUF, no PSUM needed);
            # row 7 via PE into the st slot freed by the final exp
            lo13 = pB.tile([P, 2, 2, HD + 1], f32, name="av13l", tag="pB")
            hi13 = pB.tile([P, 2, 2, HD + 1], f32, name="av13h", tag="pB")

            tp67 = {}

            def tail_row(r):
                av_norm(1, 3, r, lo13, hi13)
                if r < 6:
                    # sync queue: a DMA issue on the scalar queue would block
                    # the ACT sequencer ~667ns mid-exp-stream; gpsimd is
                    # SWDGE (no xbar access)
                    nc.sync.dma_start_transpose(
                        out=yT_t[3][:, P * r : P * (r + 1)],
                        in_=yn_t[r][:, 3],
                    )
                else:
                    # rows 6/7: PE transpose into the st slot freed by the
                    # final exp (faster than the ~1.2us xbar DMA round trip)
                    if "t" not in tp67:
                        tp67["t"] = pbig.tile([P, 2, P], f16, name="tp67", tag="pbig")
                    tp = tp67["t"][:, r - 6, :]
                    nc.tensor.matmul(
                        tp, lhsT=yn_t[r][:, 3], rhs=ident, is_transpose=True
                    )
                    nc.vector.tensor_copy(yT_t[3][:, P * r : P * (r + 1)], tp)
                nc.tensor.matmul(
                    po_tail[r], lhsT=yT_t[3][:, P * r : P * (r + 1)],
                    rhs=wp_c[3][:, 0, :], start=False, stop=True,
                )
                ot = ot_pool.tile([P, C], f16, name="ot", tag="ot")
                nc.scalar.activation(ot, po_tail[r], AF.Copy)
                eng = (nc.sync, nc.gpsimd, nc.sync, nc.scalar)[r - 4]
                eng.dma_start(out=out[P * r : P * (r + 1), :], in_=ot)

            for J in range(6):
                av_unit(1, 3, J, lo13, hi13)
            tail_row(4)
            po67 = pbig.tile([P, 2, 512], f32, name="po67", tag="pbig")
            po_tail[6] = po67[:, 0, :]
            po_tail[7] = po67[:, 1, :]
            prefire(6)
            prefire(7)
            av_unit(1, 3, 6, lo13, hi13)
            tail_row(5)
            av_unit(1, 3, 7, lo13, hi13)
            tail_row(6)
            tail_row(7)

    nc.compile()
    return nc


def _prep_inputs(x, Wq, bq, Wk, bk, Wv, bv, Wp, bp):
    """Host-side prep: transposes, bias folding, mask tables. Returns in_maps."""
    f16 = np.float16
    wqT = np.ascontiguousarray(Wq.T).astype(f16)
    wkT = np.ascontiguousarray(Wk.T).astype(f16)
    wvT = np.ascontiguousarray(Wv.T).astype(f16)
    wpT = np.ascontiguousarray(Wp.T).astype(f16)

    def mpack(wT, m):  # [P, KT*P]: (p, k*128+j) -> wT[128k+p, 128m+j]
        return np.ascontiguousarray(
            wT.reshape(C // P, P, C)[:, :, P * m : P * (m + 1)].transpose(1, 0, 2)
        ).reshape(P, C)

    bq_pp = np.ascontiguousarray(bq.astype(np.float32).reshape(C // P, P).T)
    # v bias folds into output bias: out = (y' + bv) @ Wp.T + bp
    bpp = (
        Wp.astype(np.float64) @ bv.astype(np.float64) + bp.astype(np.float64)
    ).astype(np.float32)
    bppb = np.broadcast_to(bpp[None, :], (P, C)).astype(f16).copy()
    ident = np.eye(P, dtype=f16)
    tri = (np.arange(P)[:, None] <= np.arange(P)[None, :]).astype(f16)  # keep j<=i
    j_idx = np.arange(P)[:, None] + P * np.arange(T // P)[None, :]
    cmask = np.where(j_idx % JD == JD - 1, np.float32(NEG), np.float32(0.0)).astype(
        np.float32
    )

    shared = {
        "wvT": wvT,
        "wpT": wpT,
        "combo16": np.ascontiguousarray(np.concatenate([tri, ident], axis=1)),
        "combo32": np.ascontiguousarray(
            np.concatenate([cmask, bq_pp], axis=1).astype(np.float32)
        ),
        "bppb": bppb,
    }
    for m in range(C // P):
        shared[f"wqm{m}"] = mpack(wqT, m)
        shared[f"wkm{m}"] = mpack(wkT, m)
    in_maps = []
    for b in range(N_CORES):
        mm = dict(shared)
        mm["xT"] = np.ascontiguousarray(x[b].T).astype(f16)
        in_maps.append(mm)
    return in_maps


def kernel(x, Wq, bq, Wk, bk, Wv, bv, Wp, bp):
    from concourse import bass_utils

    x = np.asarray(x, dtype=np.float32)
    if "nc" not in _CACHE:
        _CACHE["nc"] = _build()
    nc = _CACHE["nc"]
    in_maps = _prep_inputs(
        x,
        np.asarray(Wq, np.float32),
        np.asarray(bq, np.float32),
        np.asarray(Wk, np.float32),
        np.asarray(bk, np.float32),
        np.asarray(Wv, np.float32),
        np.asarray(bv, np.float32),
        np.asarray(Wp, np.float32),
        np.asarray(bp, np.float32),
    )
    res = bass_utils.run_bass_kernel_spmd(nc, in_maps, core_ids=list(range(N_CORES)))
    return np.stack(
        [res.results[b]["out"].astype(np.float32) for b in range(N_CORES)], axis=0
    )
